# revision 2
# baseline (speedup 1.0000x reference)
"""Trainium2 Bass kernel for nn_DeepSSM: LSTM over [B=256, T=2048, obs=32] -> [B, T, 64].

Strategy
--------
Data-parallel: batch 256 -> 8 cores x 32. Per core, the 32-batch is split into
G=2 independent 16-batch chains that are software-pipelined to hide the
per-step dependency latency of the recurrence.

Everything on-chip runs in a "transposed" layout: gates live in PSUM as
[gate_idx (partitions), batch (free)], hidden/cell state as [hid, batch].
Gate columns are permuted into two 128-wide chunks: chunk1 = [i; g],
chunk2 = [f; o], and the i/f/o weight columns are pre-scaled by 0.5 so that a
single Tanh activation serves all four gates (sigmoid(x) = (1+tanh(x/2))/2).

Per 16-step window and chain, one PSUM bank holds the gate pre-activations:
cols 0:256 = chunk1 (tau-major), cols 256:512 = chunk2. Two x-projection
matmuls fill it (start=True on the first; the second accumulates onto the
bank's pending-zero region; an explicit no-sync dep keeps their order), then
per-step recurrent matmuls accumulate Wh*h. The bias rides a ones-row of x.

x is staged in a never-reused persistent SBUF region (64KB/partition per
chain) so the per-window x DMAs carry no data waits: the restrictive
DIRECT2D DMA fast path allows only the queue semaphore.

Per chain and timestep (stock ops only - custom DVE ops don't compile with
this walrus, and two-SBUF-input DVE ops must share a base partition):
  PE   : 2 matmuls (Wh_cA/Wh_cB @ h') accumulating onto the x-projection.
  ACT  : 1 tanh over both gate chunks (interleaved output); 1 tanh(0.5*y)
         for the cell state (y = 2c tracked to fold the sigmoid halves).
  DVE  : rebase copy of the o/g half to partition 0; paired mult+add
         -> S = [(1+t_f)y | (1+t_i)t_g] interleaved; pairwise
         tensor_tensor_scan (d0 = [0, .5]) -> y' = S_i + S_f/2; then
         h' = 2h = (1+t_o)tanh(c') via mult+add (Wh pre-halved on host,
         output halved on host).

Host side pre-transposes x and post-transposes the output, so the device
never transposes anything.
"""

import os
import numpy as np
import ml_dtypes

BF16 = ml_dtypes.bfloat16

OBS = 32
HID = 64
T_FULL = 2048
B_FULL = 256
N_CORES = 8
BPC = B_FULL // N_CORES  # 32 batch per core
G = int(os.environ.get("LSTM_G", "2"))   # chains per core
BG = BPC // G            # batch per chain
WIN = 512 // (2 * BG)    # timesteps per PSUM window (WIN * 2 * BG = 512 cols)
KA = OBS + 1             # x rows incl ones-row

_NC_CACHE = {}


# --------------------------------------------------------------------------
# Custom DVE ops
# --------------------------------------------------------------------------
_OPS_REGISTERED = False
PAIRPROD = None  # out = s0 * (1 + in0) * in1
TANHPOLY = None  # out = clamp(x*(s0 + s1*x^2 + imm2*x^4), -1, 1)  ~ tanh(x)
# Minimax fit of tanh via output-clamped odd quintic (max abs err ~1.9e-2).
TANH_C = (0.9312120465782658, -0.1763841940228923, 0.015448984744725808)


def _register_dve_ops():
    global _OPS_REGISTERED, PAIRPROD, TANHPOLY
    if _OPS_REGISTERED:
        return
    import concourse.dve_ops as dve_ops
    from concourse.dve_ops import DveOp
    from concourse.dve_spec import (Spec, Src0, Src1, C0, C1, C2, One, Zero,
                                    minn, maxx, sq, lower, _has_src1)
    from concourse.dve_uop import DveOpSpec

    def _make(name, spec):
        existing = next((op for op in dve_ops.OPS if op.name == name), None)
        if existing is not None:
            return existing
        row = dve_ops._CUSTOM_DVE_ROW_BASE + len(dve_ops.OPS)
        dve_ops._SUB_OPCODE_FOR_NAME[name] = row
        shas = {}
        for ver in ("v3", "v4"):
            s = DveOpSpec(name=name, opcode=row, uops=lower(spec, ver=ver),
                          rd1_en=_has_src1(spec))
            shas[ver] = s.sha(ver)
        op = DveOp(name, spec, subdim=False, uops_sha=shas)
        dve_ops.OPS.append(op)
        dve_ops.CUSTOM_DVE_SPECS[name] = spec
        return op

    PAIRPROD = _make("LSTM_PAIRPROD_ANT", Spec(
        body=(Src0 + One) * Src1 * C0,
        reference=lambda in0, in1, s0, s1, imm2: (
            (in0.astype(np.float32) + 1.0)
            * np.asarray(in1, np.float32).reshape(in0.shape) * s0
        ),
    ))

    z = sq(Src0)
    p = Src0 * (C0 + z * (C1 + z * C2))
    TANHPOLY = _make("LSTM_TANHPOLY_ANT", Spec(
        body=maxx(minn(p, One), Zero - One),
        reference=lambda in0, in1, s0, s1, imm2: np.clip(
            in0.astype(np.float32)
            * (s0 + in0.astype(np.float32) ** 2
               * (s1 + in0.astype(np.float32) ** 2 * imm2)), -1.0, 1.0),
    ))
    _OPS_REGISTERED = True


# --------------------------------------------------------------------------
# Device program
# --------------------------------------------------------------------------
def build_nc(t_steps=T_FULL, n_dve_tanh=int(os.environ.get("LSTM_DVE_TANH", "0"))):
    """Build the Bass program for one core (all cores run the same NEFF).

    n_dve_tanh: number of chains (0..G) whose cell-state tanh runs as a
    polynomial approximation on the Vector engine instead of ScalarE.
    """
    _register_dve_ops()
    import concourse.bass as bass
    import concourse.tile as tile
    import concourse.mybir as mybir
    from concourse.tile import add_dep_helper

    f32 = mybir.dt.float32
    bf16 = mybir.dt.bfloat16
    TANH = mybir.ActivationFunctionType.Tanh

    n_win = t_steps // WIN
    SW = 2 * BG              # bank columns per step across both chunks
    NW = WIN * BG            # bank columns per chunk per window (256)
    nc = bass.Bass("TRN2", debug=False, num_devices=N_CORES,
                   enable_partition_id=False)

    # DRAM I/O (per core). x: [KA, T, BG] per chain ([x; ones] rows).
    x_dram = [nc.dram_tensor(f"x{g}", [KA, t_steps, BG], bf16,
                             kind="ExternalInput") for g in range(G)]
    # All weights in one tensor/DMA: cols 0:128 = wx_c1, 128:256 = wx_c2
    # (rows 0:KA), 256:384 = wh_c1, 384:512 = wh_c2 (rows 0:64).
    wcat = nc.dram_tensor("wcat", [HID, 512], bf16, kind="ExternalInput")
    out_dram = [nc.dram_tensor(f"h{g}", [HID, t_steps, BG], bf16,
                               kind="ExternalOutput") for g in range(G)]

    with tile.TileContext(nc) as tc:
        from contextlib import ExitStack
        ctx = ExitStack()
        with ctx:
            wpool = ctx.enter_context(tc.tile_pool(name="weights", bufs=1))
            tpool = [ctx.enter_context(tc.tile_pool(name=f"T{g}", bufs=6))
                     for g in range(G)]
            wprod = [ctx.enter_context(tc.tile_pool(name=f"W{g}", bufs=4))
                     for g in range(G)]
            tcpool = [ctx.enter_context(tc.tile_pool(name=f"tc{g}", bufs=4))
                      for g in range(G)]
            hpool = [ctx.enter_context(tc.tile_pool(name=f"h{g}", bufs=3))
                     for g in range(G)]
            bankp = [ctx.enter_context(
                tc.tile_pool(name=f"psum{g}", bufs=2, space="PSUM"))
                for g in range(G)]

            w_all = wpool.tile([HID, 512], bf16)
            nc.sync.dma_start(w_all[:, :], wcat[:, :])
            wx1_ap = w_all[0:KA, 0:128]
            wx2_ap = w_all[0:KA, 128:256]
            wh1_ap = w_all[:, 256:384]
            wh2_ap = w_all[:, 384:512]
            # PE observes the weights DMA once so no later matmul needs a
            # sync-wait slot for it.
            nc.tensor.ldweights(wh1_ap)

            # Never-reused x staging region: per-window DMAs into distinct
            # slices carry no data waits (DIRECT2D DMAs only get one).
            xreg = [nc.alloc_sbuf_tensor(f"xreg{g}", [KA, t_steps * BG], bf16)
                    for g in range(G)]

            # Scan multiplier pattern [0, 0.5, 0, 0.5, ...]: resets the scan
            # state at each pair's first element, halves it at the second.
            scanc_d = nc.dram_tensor("scanc", [HID, SW], f32,
                                     kind="ExternalInput")
            scanc = wpool.tile([HID, SW], f32)
            nc.sync.dma_start(scanc[:, :], scanc_d[:, :])

            EXT = 2 * BG      # T-tile ext region width (scan out, y at odds)
            h_prev = []
            T_cur = []
            banks = [[None, None] for _ in range(G)]
            h_win = [None] * G

            for g in range(G):
                h0 = hpool[g].tile([HID, BG], bf16, tag="hinit")
                nc.vector.memset(h0[:, :], 0.0)
                h_prev.append(h0[:, :])
                t0 = tpool[g].tile([128, 3 * EXT], f32)
                nc.vector.memset(t0[0:64, 0:EXT], 0.0)  # y_0 = 2*c_0 = 0
                T_cur.append(t0)

            def start_window(g, w):
                """One DMA + two ordered matmuls: project x into a bank."""
                xw = xreg[g][:][:, w * NW:(w + 1) * NW]
                src = x_dram[g][:, w * WIN:(w + 1) * WIN, :]
                nc.sync.dma_start(xw, src.rearrange("p t b -> p (t b)"))
                bank = bankp[g].tile([128, 2 * NW], f32)
                mm1 = nc.tensor.matmul(bank[:, 0:NW], lhsT=wx1_ap, rhs=xw,
                                       start=True, stop=False,
                                       skip_group_check=True)
                mm2 = nc.tensor.matmul(bank[:, NW:2 * NW], lhsT=wx2_ap,
                                       rhs=xw, start=False, stop=False,
                                       skip_group_check=True)
                # Keep the bank-clearing mm first; same engine, no sem.
                add_dep_helper(mm2.ins, mm1.ins, sync=False,
                               reason="xproj order after bank clear")
                banks[g][w % 2] = bank

            for g in range(G):
                start_window(g, 0)

            for w in range(n_win):
                for g in range(G):
                    if w + 1 < n_win:
                        start_window(g, w + 1)
                    h_win[g] = hpool[g].tile([HID, WIN * BG], bf16,
                                             name=f"hwin{g}_{w}", tag="hwin")
                for tau in range(WIN):
                    for g in range(G):
                        bank = banks[g][w % 2]
                        cA = bank[:, tau * BG:(tau + 1) * BG]
                        cB = bank[:, NW + tau * BG:NW + (tau + 1) * BG]
                        last = tau == WIN - 1
                        nc.tensor.matmul(cA, lhsT=wh1_ap, rhs=h_prev[g],
                                         start=False, stop=False,
                                         skip_group_check=True)
                        nc.tensor.matmul(cB, lhsT=wh2_ap, rhs=h_prev[g],
                                         start=False, stop=last,
                                         skip_group_check=True)
                        Tc = T_cur[g]
                        # T layout (all pair math at base partition 0):
                        # cols 0:EXT        p<64: ext (y=2c' at odd slots)
                        # cols EXT:2EXT     p<64: copy of o@even/g@odd half
                        # cols 2EXT:3EXT    tanh(gates) interleaved
                        #   (p<64: f@even, i@odd; p>=64: o@even, g@odd)
                        act_in = bank[:, :].rearrange(
                            "p (c n) -> p c n", c=2)[:, :,
                                                     tau * BG:(tau + 1) * BG]
                        act_out = Tc[:, 2 * EXT:3 * EXT].rearrange(
                            "p (n c) -> p c n", c=2)
                        nc.scalar.activation(act_out, act_in, TANH)
                        # rebase the o/g half to partition 0 (walrus forbids
                        # two-SBUF-input ops with differing base partitions)
                        nc.vector.tensor_copy(Tc[0:64, EXT:2 * EXT],
                                              Tc[64:128, 2 * EXT:3 * EXT])

                        Tn = tpool[g].tile([128, 3 * EXT], f32)
                        Mt = wprod[g].tile([HID, SW], f32, tag="m")
                        St = wprod[g].tile([HID, SW], f32, tag="s")
                        # pairs: f<->y (=2c), i<->g
                        src0 = Tc[0:64, 2 * EXT:3 * EXT].rearrange(
                            "p (n c) -> p c n", c=2)          # f's then i's
                        src1 = Tc[0:64, 0:2 * EXT].rearrange(
                            "p (b n c) -> p b c n", b=2, c=2)[:, :, 1, :]
                        nc.vector.tensor_tensor(Mt[:, :], src0, src1,
                                                mybir.AluOpType.mult)
                        nc.vector.tensor_tensor(
                            St[:, :].rearrange("p (n c) -> p c n", c=2),
                            Mt[:, :], src1, mybir.AluOpType.add)
                        # y' = S_i + 0.5*S_f via pairwise scan (d0=[0,.5])
                        nc.vector.tensor_tensor_scan(
                            Tn[0:64, 0:EXT], scanc[:, :], St[:, :], 0.0,
                            mybir.AluOpType.mult, mybir.AluOpType.add)
                        tct = tcpool[g].tile([HID, BG], f32)
                        nc.scalar.activation(
                            tct[:, :],
                            Tn[0:64, 0:EXT].rearrange(
                                "p (n c) -> p c n", c=2)[:, 1, :],
                            TANH, scale=0.5)
                        # h' = 2h = (1+t_o)*tanh(c'); Wh is pre-halved and
                        # the host halves the output.
                        h_sl = h_win[g][:, tau * BG:(tau + 1) * BG]
                        t_o = Tc[0:64, EXT:2 * EXT].rearrange(
                            "p (n c) -> p c n", c=2)[:, 0, :]
                        m2 = tcpool[g].tile([HID, BG], f32, tag="m2")
                        nc.vector.tensor_tensor(m2[:, :], t_o, tct[:, :],
                                                mybir.AluOpType.mult)
                        nc.vector.tensor_tensor(h_sl, m2[:, :], tct[:, :],
                                                mybir.AluOpType.add)
                        h_prev[g] = h_sl
                        T_cur[g] = Tn
                for g in range(G):
                    dst = out_dram[g][:, w * WIN:(w + 1) * WIN, :]
                    nc.sync.dma_start(dst.rearrange("p t b -> p (t b)"),
                                      h_win[g][:, :])
    return nc


def _split_waits(nc, mybir, nmax=1):
    """This walrus accepts only one sync-wait per instruction: move excess
    waits onto preceding same-engine NOPs."""
    fn = nc.m.functions[0]
    for bb in fn.blocks:
        newlist = []
        for ins in bb.instructions:
            si = getattr(ins, "sync_info", None)
            if si is not None and si.on_wait and len(si.on_wait) > nmax:
                waits = list(si.on_wait)
                while len(waits) > nmax:
                    chunk, waits = waits[:nmax], waits[nmax:]
                    nop = mybir.InstNoOp(
                        name=nc.get_next_instruction_name(), ins=[], outs=[])
                    nop.engine = ins.engine
                    nop.sync_info = mybir.SyncInfo(on_wait=chunk, on_update=[])
                    newlist.append(nop)
                si.on_wait = waits
            newlist.append(ins)
        bb.instructions[:] = newlist


# --------------------------------------------------------------------------
# Host-side weight/input prep
# --------------------------------------------------------------------------
def _prep_weights(Wx, Wh, b):
    """Permute gate columns into chunks [i;g] and [f;o]; scale i/f/o by 0.5;
    fold the bias into an extra row of Wx; stack everything into wcat."""
    H = HID
    idx_i = np.arange(0, H)
    idx_f = np.arange(H, 2 * H)
    idx_g = np.arange(2 * H, 3 * H)
    idx_o = np.arange(3 * H, 4 * H)
    scale = np.ones(4 * H, np.float32)
    scale[np.concatenate([idx_i, idx_f, idx_o])] = 0.5
    Wxs = (np.asarray(Wx, np.float32) * scale)
    Whs = (np.asarray(Wh, np.float32) * scale)
    bs = (np.asarray(b, np.float32) * scale)
    Wxa = np.concatenate([Wxs, bs[None, :]], axis=0)  # [KA, 256]
    c1 = np.concatenate([idx_i, idx_g])
    c2 = np.concatenate([idx_f, idx_o])
    wcat = np.zeros((HID, 512), np.float32)
    wcat[0:KA, 0:128] = Wxa[:, c2]      # chunk A = [f; o]
    wcat[0:KA, 128:256] = Wxa[:, c1]    # chunk B = [i; g]
    # Recurrent weights additionally halved: the device recurrence carries
    # h' = 2h (the host halves the output), so Wh_dev = Wh_scaled / 2.
    wcat[:, 256:384] = Whs[:, c2] * 0.5
    wcat[:, 384:512] = Whs[:, c1] * 0.5
    return wcat.astype(BF16)


def _prep_x(y_core):
    """y_core [BPC, T, OBS] fp32 -> per chain [KA, T, BG] bf16 ([x; 1])."""
    t_steps = y_core.shape[1]
    xt = y_core.transpose(2, 1, 0)  # [OBS, T, BPC]
    out = []
    for g in range(G):
        xa = np.empty((KA, t_steps, BG), np.float32)
        xa[0:OBS] = xt[:, :, g * BG:(g + 1) * BG]
        xa[OBS] = 1.0
        out.append(np.ascontiguousarray(xa.astype(BF16)))
    return out


def kernel(y, Wx, Wh, b):
    from concourse.bass_utils import run_bass_kernel_spmd

    y = np.asarray(y)
    t_steps = y.shape[1]
    wcat = _prep_weights(Wx, Wh, b)

    key = t_steps
    if key not in _NC_CACHE:
        import concourse.mybir as mybir
        nc = build_nc(t_steps)
        _split_waits(nc, mybir)   # CoreSim can't run the split form
        _NC_CACHE[key] = nc
    nc = _NC_CACHE[key]

    scanc = np.zeros((HID, 2 * BG), np.float32)
    scanc[:, 1::2] = 0.5
    in_maps = []
    for c in range(N_CORES):
        xs = _prep_x(y[c * BPC:(c + 1) * BPC])
        m = {"wcat": wcat, "scanc": scanc}
        for g in range(G):
            m[f"x{g}"] = xs[g]
        in_maps.append(m)

    globals()["_LAST_IN_MAPS"] = in_maps
    res = run_bass_kernel_spmd(
        nc, in_maps, core_ids=list(range(N_CORES)),
        trace=bool(int(os.environ.get("LSTM_TRACE", "0"))))

    out = np.empty((B_FULL, t_steps, HID), np.float32)
    for c in range(N_CORES):
        for g in range(G):
            hg = res.results[c][f"h{g}"].astype(np.float32)  # [HID, T, BG]
            out[c * BPC + g * BG:c * BPC + (g + 1) * BG] = (
                hg.transpose(2, 1, 0) * 0.5)
    globals()["_LAST_RESULT"] = res
    return out



# revision 19
# speedup vs baseline: 5.6776x; 5.6776x over previous
"""Trainium2 Bass kernel for nn_DeepSSM: LSTM over [B=256, T=2048, obs=32] -> [B, T, 64].

Strategy
--------
Data-parallel: batch 256 -> 8 cores x 32. Per core, the 32-batch is split into
G=2 independent 16-batch chains that are software-pipelined to hide the
per-step dependency latency of the recurrence.

Everything on-chip runs in a "transposed" layout: gates live in PSUM as
[gate_idx (partitions), batch (free)], hidden/cell state as [hid, batch].
Gate columns are permuted into two 128-wide chunks: chunk1 = [i; g],
chunk2 = [f; o], and the i/f/o weight columns are pre-scaled by 0.5 so that a
single Tanh activation serves all four gates (sigmoid(x) = (1+tanh(x/2))/2).

Per 16-step window and chain, one PSUM bank holds the gate pre-activations:
cols 0:256 = chunk1 (tau-major), cols 256:512 = chunk2. Two x-projection
matmuls fill it (start=True on the first; the second accumulates onto the
bank's pending-zero region; an explicit no-sync dep keeps their order), then
per-step recurrent matmuls accumulate Wh*h. The bias rides a ones-row of x.

x is staged in a never-reused persistent SBUF region (64KB/partition per
chain) so the per-window x DMAs carry no data waits: the restrictive
DIRECT2D DMA fast path allows only the queue semaphore.

Per chain and timestep (stock ops only - custom DVE ops don't compile with
this walrus, and two-SBUF-input DVE ops must share a base partition):
  PE   : 2 matmuls (Wh_cA/Wh_cB @ h') accumulating onto the x-projection.
  ACT  : 1 tanh over both gate chunks (interleaved output); 1 tanh(0.5*y)
         for the cell state (y = 2c tracked to fold the sigmoid halves).
  DVE  : rebase copy of the o/g half to partition 0; paired mult+add
         -> S = [(1+t_f)y | (1+t_i)t_g] interleaved; pairwise
         tensor_tensor_scan (d0 = [0, .5]) -> y' = S_i + S_f/2; then
         h' = 2h = (1+t_o)tanh(c') via mult+add (Wh pre-halved on host,
         output halved on host).

Host side pre-transposes x and post-transposes the output, so the device
never transposes anything.
"""

import os
import numpy as np
import ml_dtypes

BF16 = ml_dtypes.bfloat16

OBS = 32
HID = 64
T_FULL = 2048
B_FULL = 256
N_CORES = 8
BPC = B_FULL // N_CORES  # 32 batch per core
G = int(os.environ.get("LSTM_G", "2"))   # chains per core
BG = BPC // G            # batch per chain
WIN = 512 // (2 * BG)    # timesteps per PSUM window (WIN * 2 * BG = 512 cols)
KA = OBS + 1             # x rows incl ones-row

_NC_CACHE = {}


# --------------------------------------------------------------------------
# Custom DVE ops
# --------------------------------------------------------------------------
_OPS_REGISTERED = False
PAIRPROD = None  # out = s0 * (1 + in0) * in1
TANHPOLY = None  # out = clamp(x*(s0 + s1*x^2 + imm2*x^4), -1, 1)  ~ tanh(x)
# Minimax fit of tanh via output-clamped odd quintic (max abs err ~1.9e-2).
TANH_C = (0.9312120465782658, -0.1763841940228923, 0.015448984744725808)


def _register_dve_ops():
    global _OPS_REGISTERED, PAIRPROD, TANHPOLY
    if _OPS_REGISTERED:
        return
    import concourse.dve_ops as dve_ops
    from concourse.dve_ops import DveOp
    from concourse.dve_spec import (Spec, Src0, Src1, C0, C1, C2, One, Zero,
                                    minn, maxx, sq, lower, _has_src1)
    from concourse.dve_uop import DveOpSpec

    def _make(name, spec):
        existing = next((op for op in dve_ops.OPS if op.name == name), None)
        if existing is not None:
            return existing
        row = dve_ops._CUSTOM_DVE_ROW_BASE + len(dve_ops.OPS)
        dve_ops._SUB_OPCODE_FOR_NAME[name] = row
        shas = {}
        for ver in ("v3", "v4"):
            s = DveOpSpec(name=name, opcode=row, uops=lower(spec, ver=ver),
                          rd1_en=_has_src1(spec))
            shas[ver] = s.sha(ver)
        op = DveOp(name, spec, subdim=False, uops_sha=shas)
        dve_ops.OPS.append(op)
        dve_ops.CUSTOM_DVE_SPECS[name] = spec
        return op

    PAIRPROD = _make("LSTM_PAIRPROD_ANT", Spec(
        body=(Src0 + One) * Src1 * C0,
        reference=lambda in0, in1, s0, s1, imm2: (
            (in0.astype(np.float32) + 1.0)
            * np.asarray(in1, np.float32).reshape(in0.shape) * s0
        ),
    ))

    z = sq(Src0)
    p = Src0 * (C0 + z * (C1 + z * C2))
    TANHPOLY = _make("LSTM_TANHPOLY_ANT", Spec(
        body=maxx(minn(p, One), Zero - One),
        reference=lambda in0, in1, s0, s1, imm2: np.clip(
            in0.astype(np.float32)
            * (s0 + in0.astype(np.float32) ** 2
               * (s1 + in0.astype(np.float32) ** 2 * imm2)), -1.0, 1.0),
    ))
    _OPS_REGISTERED = True


# --------------------------------------------------------------------------
# Device program
# --------------------------------------------------------------------------
def build_nc(t_steps=T_FULL, n_dve_tanh=int(os.environ.get("LSTM_DVE_TANH", "0"))):
    """Build the Bass program for one core (all cores run the same NEFF).

    n_dve_tanh: number of chains (0..G) whose cell-state tanh runs as a
    polynomial approximation on the Vector engine instead of ScalarE.
    """
    _register_dve_ops()
    import concourse.bass as bass
    import concourse.tile as tile
    import concourse.mybir as mybir
    from concourse.tile import add_dep_helper

    f32 = mybir.dt.float32
    bf16 = mybir.dt.bfloat16
    TANH = mybir.ActivationFunctionType.Tanh

    SKIP = set(os.environ.get("LSTM_SKIP", "").split(",")) - {""}

    n_win = t_steps // WIN
    SW = 2 * BG              # bank columns per step across both chunks
    NW = WIN * BG            # bank columns per chunk per window (256)
    nc = bass.Bass("TRN2", debug=False, num_devices=N_CORES,
                   enable_partition_id=False)

    # DRAM I/O (per core). x: [KA, T, BG] per chain ([x; ones] rows).
    x_dram = [nc.dram_tensor(f"x{g}", [KA, t_steps, BG], bf16,
                             kind="ExternalInput") for g in range(G)]
    # All weights in one tensor/DMA: cols 0:128 = wx_c1, 128:256 = wx_c2
    # (rows 0:KA), 256:384 = wh_c1, 384:512 = wh_c2 (rows 0:64).
    wcat = nc.dram_tensor("wcat", [HID, 512], bf16, kind="ExternalInput")
    out_dram = [nc.dram_tensor(f"h{g}", [HID, t_steps, BG], bf16,
                               kind="ExternalOutput") for g in range(G)]

    with tile.TileContext(nc) as tc:
        from contextlib import ExitStack
        ctx = ExitStack()
        with ctx:
            wpool = ctx.enter_context(tc.tile_pool(name="weights", bufs=1))
            tpool = [ctx.enter_context(tc.tile_pool(name=f"T{g}", bufs=6))
                     for g in range(G)]
            wprod = [ctx.enter_context(tc.tile_pool(name=f"W{g}", bufs=4))
                     for g in range(G)]
            tcpool = [ctx.enter_context(tc.tile_pool(name=f"tc{g}", bufs=4))
                      for g in range(G)]
            hpool = [ctx.enter_context(tc.tile_pool(name=f"h{g}", bufs=3))
                     for g in range(G)]
            bankp = [ctx.enter_context(
                tc.tile_pool(name=f"psum{g}", bufs=2, space="PSUM"))
                for g in range(G)]

            w_all = wpool.tile([HID, 512], bf16)
            nc.sync.dma_start(w_all[:, :], wcat[:, :])
            wx1_ap = w_all[0:KA, 0:128]
            wx2_ap = w_all[0:KA, 128:256]
            wh1_ap = w_all[:, 256:384]
            wh2_ap = w_all[:, 384:512]
            # PE observes the weights DMA once so no later matmul needs a
            # sync-wait slot for it.
            nc.tensor.ldweights(wh1_ap)

            # Never-reused x staging region: per-window DMAs into distinct
            # slices carry no data waits (DIRECT2D DMAs only get one).
            xreg = [nc.alloc_sbuf_tensor(f"xreg{g}", [KA, t_steps * BG], bf16)
                    for g in range(G)]

            # Scan multiplier pattern [0, 0.5, 0, 0.5, ...]: resets the scan
            # state at each pair's first element, halves it at the second.
            scanc_d = nc.dram_tensor("scanc", [HID, SW], f32,
                                     kind="ExternalInput")
            scanc = wpool.tile([HID, SW], f32)
            nc.sync.dma_start(scanc[:, :], scanc_d[:, :])

            EXT = 2 * BG      # T-tile ext region width (scan out, y at odds)
            h_prev = []
            T_cur = []
            banks = [[None, None] for _ in range(G)]
            h_win = [None] * G
            bankB = [None] * G
            h_wide = [None] * G
            if "bankB" in SKIP:
                bpoolB = [ctx.enter_context(
                    tc.tile_pool(name=f"psumB{g}", bufs=1, space="PSUM"))
                    for g in range(G)]
                for g in range(G):
                    bankB[g] = bpoolB[g].tile([128, WIN * BG], f32,
                                              name=f"bankBt{g}")
            if "mmwide" in SKIP:
                hwpool = [ctx.enter_context(tc.tile_pool(name=f"hw{g}", bufs=1))
                          for g in range(G)]
                for g in range(G):
                    h_wide[g] = hwpool[g].tile([HID, SW], bf16,
                                               name=f"hwt{g}")
                    nc.vector.memset(h_wide[g][:, :], 0.0)

            for g in range(G):
                h0 = hpool[g].tile([HID, BG], bf16, tag="hinit")
                nc.vector.memset(h0[:, :], 0.0)
                h_prev.append(h0[:, :])
                t0 = tpool[g].tile([128, 3 * EXT], f32)
                nc.vector.memset(t0[0:64, 0:EXT], 0.0)  # y_0 = 2*c_0 = 0
                T_cur.append(t0)

            def start_window(g, w):
                """One DMA + two ordered matmuls: project x into a bank."""
                xw = xreg[g][:][:, w * NW:(w + 1) * NW]
                src = x_dram[g][:, w * WIN:(w + 1) * WIN, :]
                nc.sync.dma_start(xw, src.rearrange("p t b -> p (t b)"))
                bank = bankp[g].tile([128, 2 * NW], f32)
                mm1 = nc.tensor.matmul(bank[:, 0:NW], lhsT=wx1_ap, rhs=xw,
                                       start=True, stop=False,
                                       skip_group_check=True)
                mm2 = nc.tensor.matmul(bank[:, NW:2 * NW], lhsT=wx2_ap,
                                       rhs=xw, start=False, stop=False,
                                       skip_group_check=True)
                # Keep the bank-clearing mm first; same engine, no sem.
                add_dep_helper(mm2.ins, mm1.ins, sync=False,
                               reason="xproj order after bank clear")
                banks[g][w % 2] = bank

            for g in range(G):
                start_window(g, 0)

            for w in range(n_win):
                for g in range(G):
                    if w + 1 < n_win:
                        start_window(g, w + 1)
                    if "mmwide" not in SKIP:
                        h_win[g] = hpool[g].tile([HID, WIN * BG], bf16,
                                                 name=f"hwin{g}_{w}", tag="hwin")
                for tau in range(WIN):
                    for g in range(G):
                        bank = banks[g][w % 2]
                        cA = bank[:, tau * BG:(tau + 1) * BG]
                        cB = bank[:, NW + tau * BG:NW + (tau + 1) * BG]
                        last = tau == WIN - 1
                        if "mmwide" in SKIP:
                            # one [128, 2BG] matmul (chunk B data wrong;
                            # timing probe only)
                            nc.tensor.matmul(
                                bank[:, tau * SW:(tau + 1) * SW],
                                lhsT=wh1_ap, rhs=h_wide[g][:, 0:SW],
                                start=False, stop=False,
                                skip_group_check=True)
                        else:
                            nc.tensor.matmul(cA, lhsT=wh1_ap, rhs=h_prev[g],
                                             start=False, stop=False,
                                             skip_group_check=True)
                        if "mm2" not in SKIP and "mmwide" not in SKIP:
                            w2 = wh1_ap if "samew" in SKIP else wh2_ap
                            if "bankB" in SKIP:
                                cB2 = bankB[g][:, tau * BG:(tau + 1) * BG]
                                nc.tensor.matmul(cB2, lhsT=w2, rhs=h_prev[g],
                                                 start=(tau == 0), stop=last,
                                                 skip_group_check=True)
                            elif "ccols" in SKIP:
                                nc.tensor.matmul(
                                    bank[:, tau * SW + BG:(tau + 1) * SW],
                                    lhsT=w2, rhs=h_prev[g],
                                    start=False, stop=last,
                                    skip_group_check=True)
                            else:
                                nc.tensor.matmul(cB, lhsT=w2, rhs=h_prev[g],
                                                 start=False, stop=last,
                                                 skip_group_check=True)
                        Tc = T_cur[g]
                        # T layout (all pair math at base partition 0):
                        # cols 0:EXT        p<64: ext (y=2c' at odd slots)
                        # cols EXT:2EXT     p<64: copy of o@even/g@odd half
                        # cols 2EXT:3EXT    tanh(gates) interleaved
                        #   (p<64: f@even, i@odd; p>=64: o@even, g@odd)
                        if "act1s" in SKIP:
                            act_in = bank[:, tau * SW:(tau + 1) * SW]
                            act_out = Tc[:, 2 * EXT:3 * EXT]
                        else:
                            act_in = bank[:, :].rearrange(
                                "p (c n) -> p c n", c=2)[:, :,
                                                         tau * BG:(tau + 1) * BG]
                            act_out = Tc[:, 2 * EXT:3 * EXT].rearrange(
                                "p (n c) -> p c n", c=2)
                        nc.scalar.activation(act_out, act_in, TANH)
                        # rebase the o/g half to partition 0 (walrus forbids
                        # two-SBUF-input ops with differing base partitions)
                        if "copy" not in SKIP:
                            nc.vector.tensor_copy(Tc[0:64, EXT:2 * EXT],
                                                  Tc[64:128, 2 * EXT:3 * EXT])

                        Tn = tpool[g].tile([128, 3 * EXT], f32)
                        Mt = wprod[g].tile([HID, SW], f32, tag="m")
                        St = wprod[g].tile([HID, SW], f32, tag="s")
                        # pairs: f<->y (=2c), i<->g
                        src0 = Tc[0:64, 2 * EXT:3 * EXT].rearrange(
                            "p (n c) -> p c n", c=2)          # f's then i's
                        src1 = Tc[0:64, 0:2 * EXT].rearrange(
                            "p (b n c) -> p b c n", b=2, c=2)[:, :, 1, :]
                        if "dve2" not in SKIP:
                            nc.vector.tensor_tensor(Mt[:, :], src0, src1,
                                                    mybir.AluOpType.mult)
                            nc.vector.tensor_tensor(
                                St[:, :].rearrange("p (n c) -> p c n", c=2),
                                Mt[:, :], src1, mybir.AluOpType.add)
                        # y' = S_i + 0.5*S_f via pairwise scan (d0=[0,.5])
                        if "scan" in SKIP:
                            nc.vector.tensor_tensor(
                                Tn[0:64, 0:EXT].rearrange(
                                    "p (n c) -> p c n", c=2)[:, 1, :],
                                St[:, :].rearrange(
                                    "p (c n) -> p c n", c=2)[:, 0, :],
                                St[:, :].rearrange(
                                    "p (c n) -> p c n", c=2)[:, 1, :],
                                mybir.AluOpType.add)
                        else:
                            nc.vector.tensor_tensor_scan(
                                Tn[0:64, 0:EXT], scanc[:, :], St[:, :], 0.0,
                                mybir.AluOpType.mult, mybir.AluOpType.add)
                        tct = tcpool[g].tile([HID, BG], f32)
                        if "act2s" in SKIP:
                            nc.scalar.activation(tct[:, :], Tn[0:64, 0:BG],
                                                 TANH, scale=0.5)
                        else:
                            nc.scalar.activation(
                                tct[:, :],
                                Tn[0:64, 0:EXT].rearrange(
                                    "p (n c) -> p c n", c=2)[:, 1, :],
                                TANH, scale=0.5)
                        # h' = 2h = (1+t_o)*tanh(c'); Wh is pre-halved and
                        # the host halves the output.
                        if "mmwide" in SKIP:
                            h_sl = h_wide[g][:, 0:BG]
                        else:
                            h_sl = h_win[g][:, tau * BG:(tau + 1) * BG]
                        t_o = Tc[0:64, EXT:2 * EXT].rearrange(
                            "p (n c) -> p c n", c=2)[:, 0, :]
                        m2 = tcpool[g].tile([HID, BG], f32, tag="m2")
                        if "hmul" in SKIP:
                            nc.vector.tensor_copy(h_sl, tct[:, :])
                        else:
                            nc.vector.tensor_tensor(m2[:, :], t_o, tct[:, :],
                                                    mybir.AluOpType.mult)
                            nc.vector.tensor_tensor(h_sl, m2[:, :], tct[:, :],
                                                    mybir.AluOpType.add)
                        h_prev[g] = h_sl
                        T_cur[g] = Tn
                for g in range(G):
                    if "mmwide" in SKIP:
                        continue
                    dst = out_dram[g][:, w * WIN:(w + 1) * WIN, :]
                    nc.sync.dma_start(dst.rearrange("p t b -> p (t b)"),
                                      h_win[g][:, :])
    return nc


def _split_waits(nc, mybir, nmax=1):
    """This walrus accepts only one sync-wait per instruction: move excess
    waits onto preceding same-engine NOPs."""
    fn = nc.m.functions[0]
    for bb in fn.blocks:
        newlist = []
        for ins in bb.instructions:
            si = getattr(ins, "sync_info", None)
            if si is not None and si.on_wait and len(si.on_wait) > nmax:
                waits = list(si.on_wait)
                while len(waits) > nmax:
                    chunk, waits = waits[:nmax], waits[nmax:]
                    nop = mybir.InstNoOp(
                        name=nc.get_next_instruction_name(), ins=[], outs=[])
                    nop.engine = ins.engine
                    nop.sync_info = mybir.SyncInfo(on_wait=chunk, on_update=[])
                    newlist.append(nop)
                si.on_wait = waits
            newlist.append(ins)
        bb.instructions[:] = newlist


# --------------------------------------------------------------------------
# V2: single block-diagonal recurrent matmul per step, real sigmoid/tanh.
#
# Gate chunks: A = [i; o] (psum partitions 0:64 = i, 64:128 = o),
#              B = [f; g] (0:64 = f, 64:128 = g).
# Step tau's gates live in psum bank cols [64*tau, 64*tau+64): first 32 cols
# chunk A, next 32 chunk B. The recurrent matmul is ONE instruction with
# lhsT = [[WhB],[WhA]] (128x128: rows 0:64 feed B-cols, 64:128 feed A-cols)
# and rhs = the h-slot [128, 64] bf16: block-A cols have h at rows 64:128,
# block-B cols have h at rows 0:64, zeros elsewhere (memset once).
#
# Per step: ACT sigmoid over all gates (chunk-interleaved out: p<64 =
# (sig_i@even, sig_f@odd), p>=64 = (sig_o@even, junk@odd)); ACT tanh(g-hat)
# -> X[0:B); DVE mult M = (sig_i*t_g, sig_f*c); DVE pairwise scan
# (d0=[0,1]) -> c' written contiguously into X_next[B:2B); ACT tanh(c') ->
# wt[64:128]; DVE h = sig_o * w -> rhs slot t+1 (A-quadrant, rows 64:128);
# Pool copy -> B-quadrant rows 0:64. h slots double as output staging.
# --------------------------------------------------------------------------
B2 = BPC                   # 32 batch per core, one chain
S2 = 256                   # steps per staging chunk
WIN2 = 8                   # steps per psum bank (8 * 64 cols = 512)


def build_nc2(t_steps=T_FULL):
    import concourse.bass as bass
    import concourse.tile as tile
    import concourse.mybir as mybir

    S2 = min(256, t_steps)

    f32 = mybir.dt.float32
    bf16 = mybir.dt.bfloat16
    TANH = mybir.ActivationFunctionType.Tanh
    SIG = mybir.ActivationFunctionType.Sigmoid
    MULT = mybir.AluOpType.mult
    ADD = mybir.AluOpType.add

    B = B2
    S = S2
    n_chunk = t_steps // S
    n_bank = S // WIN2          # banks per chunk
    nc = bass.Bass("TRN2", debug=False, num_devices=N_CORES,
                   enable_partition_id=False)

    xcat = nc.dram_tensor("xcat", [2 * KA, t_steps, B], bf16,
                          kind="ExternalInput")
    wall = nc.dram_tensor("wall", [128, 256], bf16, kind="ExternalInput")
    hout = nc.dram_tensor("hout", [HID, t_steps, B], bf16,
                          kind="ExternalOutput")

    with tile.TileContext(nc) as tc:
        from contextlib import ExitStack
        ctx = ExitStack()
        with ctx:
            wpool = ctx.enter_context(tc.tile_pool(name="w", bufs=1))
            tpool = ctx.enter_context(tc.tile_pool(name="T", bufs=4))
            xpool = ctx.enter_context(tc.tile_pool(name="X", bufs=4))
            mpool = ctx.enter_context(tc.tile_pool(name="M", bufs=4))
            wtpool = ctx.enter_context(tc.tile_pool(name="wt", bufs=4))
            rhsp = ctx.enter_context(tc.tile_pool(name="rhs", bufs=1))
            xsp = ctx.enter_context(tc.tile_pool(name="xs", bufs=1))
            bankp = ctx.enter_context(
                tc.tile_pool(name="bank", bufs=2, space="PSUM"))

            w_all = wpool.tile([128, 256], bf16)
            nc.sync.dma_start(w_all[:, :], wall[:, :])
            wh_ap = w_all[:, 0:128]          # block-diag [WhB; WhA]
            wx_ap = w_all[0:2 * KA, 128:128 + 128]  # [66, 128]

            scanc = wpool.tile([HID, 2 * B], f32)
            nc.vector.memset(scanc[:, :].rearrange(
                "p (b c) -> p c b", c=2)[:, 0, :], 0.0)
            nc.vector.memset(scanc[:, :].rearrange(
                "p (b c) -> p c b", c=2)[:, 1, :], 1.0)

            # persistent double-buffered h-slot / x staging regions
            rhs_bufs = [rhsp.tile([128, S * 2 * B], bf16, name=f"rhsb{i}")
                        for i in range(2)]
            xs_bufs = [xsp.tile([2 * KA, S * 2 * B], bf16, name=f"xsb{i}")
                       for i in range(2)]
            for i in range(2):
                nc.vector.memset(rhs_bufs[i][:, :], 0.0)
                nc.gpsimd.memset(xs_bufs[i][:, :], 0.0)

            def stage_x(c):
                """DMA chunk c's x into staging buffer c%2 (two quadrants)."""
                buf = xs_bufs[c % 2]
                t0 = c * S
                # chunk B rows (0:33) -> odd 32-col blocks (cols 32:64 mod 64)
                dstB = buf[0:KA, :].rearrange(
                    "p (s c) -> p s c", c=2 * B)[:, :, B:2 * B]
                nc.sync.dma_start(
                    dstB, xcat[0:KA, t0:t0 + S, :])
                # chunk A rows (33:66) -> even 32-col blocks
                dstA = buf[KA:2 * KA, :].rearrange(
                    "p (s c) -> p s c", c=2 * B)[:, :, 0:B]
                nc.sync.dma_start(
                    dstA, xcat[KA:2 * KA, t0:t0 + S, :])

            def out_dma(c):
                """DMA chunk c's h (slots 1..S of buffer c%2 hold
                h_{cS}..h_{cS+S-1}... slot j holds h_{cS+j-1}."""
                buf = rhs_bufs[c % 2]
                # slots 1..S-1 -> h_{cS..cS+S-2}; h values sit in the
                # A-quadrant: rows 64:128, first 32 cols of each slot.
                src = buf[64:128, :].rearrange(
                    "p (s c) -> p s c", c=2 * B)[:, 1:S, 0:B]
                nc.sync.dma_start(hout[:, c * S:c * S + S - 1, :], src)

            def out_dma_tail(c):
                """h_{cS+S-1} lands in buffer (c+1)%2 slot 0."""
                buf = rhs_bufs[(c + 1) % 2]
                src = buf[64:128, :].rearrange(
                    "p (s c) -> p s c", c=2 * B)[:, 0, 0:B]
                nc.sync.dma_start(hout[:, c * S + S - 1, :], src)

            stage_x(0)
            if n_chunk > 1:
                stage_x(1)

            X_cur = xpool.tile([HID, 2 * B], f32, name="X0")
            nc.vector.memset(X_cur[:, :], 0.0)   # c_{-1} = 0

            for c in range(n_chunk):
                rbuf = rhs_bufs[c % 2]
                xbuf = xs_bufs[c % 2]
                for k in range(n_bank):
                    bank = bankp.tile([128, 512], f32)
                    nc.tensor.matmul(
                        bank[:, :], lhsT=wx_ap,
                        rhs=xbuf[:, k * 512:(k + 1) * 512],
                        start=True, stop=False, skip_group_check=True)
                    for j in range(WIN2):
                        tau = k * WIN2 + j          # chunk-local step
                        t = c * S + tau             # global step
                        glob_next = t + 1
                        nrbuf = rhs_bufs[(glob_next // S) % 2]
                        nslot = glob_next % S
                        rhs_sl = rbuf[:, tau * 2 * B:(tau + 1) * 2 * B]
                        nc.tensor.matmul(
                            bank[:, j * 2 * B:(j + 1) * 2 * B],
                            lhsT=wh_ap, rhs=rhs_sl,
                            start=False, stop=(j == WIN2 - 1),
                            skip_group_check=True)
                        # T contiguous: [0:B)=chunk A (sig_i p<64, sig_o
                        # p>=64), [B:2B)=chunk B (sig_f p<64, junk p>=64)
                        Tt = tpool.tile([128, 2 * B], f32)
                        nc.scalar.activation(
                            Tt[:, :], bank[:, j * 2 * B:(j + 1) * 2 * B], SIG)
                        # t_g -> X even slots (X: t_g@even, c@odd)
                        nc.scalar.activation(
                            X_cur[:, :].rearrange(
                                "p (b c) -> p c b", c=2)[:, 0, :],
                            bank[64:128, j * 2 * B + B:(j + 1) * 2 * B],
                            TANH)
                        # M = (sig_i*t_g @even, sig_f*c @odd)
                        Mt = mpool.tile([HID, 2 * B], f32)
                        nc.vector.tensor_tensor(
                            Mt[:, :],
                            Tt[0:64, :].rearrange("p (c b) -> p b c", c=2),
                            X_cur[:, :], MULT)
                        # c' = S_i + S_f -> X_next odd slots (scan d0=[0,1])
                        X_next = xpool.tile([HID, 2 * B], f32)
                        nc.vector.tensor_tensor_scan(
                            X_next[:, :], scanc[:, :], Mt[:, :],
                            0.0, MULT, ADD)
                        wt = wtpool.tile([128, B], f32)
                        nc.scalar.activation(
                            wt[64:128, :],
                            X_next[:, :].rearrange(
                                "p (b c) -> p c b", c=2)[:, 1, :],
                            TANH)
                        # h = sig_o * w -> next slot A-quadrant (rows 64:128)
                        nsl = nrbuf[:, nslot * 2 * B:(nslot + 1) * 2 * B]
                        nc.vector.tensor_tensor(
                            nsl[64:128, 0:B], Tt[64:128, 0:B],
                            wt[64:128, :], MULT)
                        # B-quadrant copy (rows 0:64, cols B:2B)
                        nc.gpsimd.tensor_copy(nsl[0:64, B:2 * B],
                                              nsl[64:128, 0:B])
                        X_cur = X_next
                if c + 2 < n_chunk:
                    stage_x(c + 2)
                out_dma(c)
                out_dma_tail(c)
    return nc


# --------------------------------------------------------------------------
# Host-side weight/input prep
# --------------------------------------------------------------------------
def _prep_weights(Wx, Wh, b):
    """Permute gate columns into chunks [i;g] and [f;o]; scale i/f/o by 0.5;
    fold the bias into an extra row of Wx; stack everything into wcat."""
    H = HID
    idx_i = np.arange(0, H)
    idx_f = np.arange(H, 2 * H)
    idx_g = np.arange(2 * H, 3 * H)
    idx_o = np.arange(3 * H, 4 * H)
    scale = np.ones(4 * H, np.float32)
    scale[np.concatenate([idx_i, idx_f, idx_o])] = 0.5
    Wxs = (np.asarray(Wx, np.float32) * scale)
    Whs = (np.asarray(Wh, np.float32) * scale)
    bs = (np.asarray(b, np.float32) * scale)
    Wxa = np.concatenate([Wxs, bs[None, :]], axis=0)  # [KA, 256]
    c1 = np.concatenate([idx_i, idx_g])
    c2 = np.concatenate([idx_f, idx_o])
    wcat = np.zeros((HID, 512), np.float32)
    wcat[0:KA, 0:128] = Wxa[:, c2]      # chunk A = [f; o]
    wcat[0:KA, 128:256] = Wxa[:, c1]    # chunk B = [i; g]
    # Recurrent weights additionally halved: the device recurrence carries
    # h' = 2h (the host halves the output), so Wh_dev = Wh_scaled / 2.
    wcat[:, 256:384] = Whs[:, c2] * 0.5
    wcat[:, 384:512] = Whs[:, c1] * 0.5
    return wcat.astype(BF16)


def _prep_x(y_core):
    """y_core [BPC, T, OBS] fp32 -> per chain [KA, T, BG] bf16 ([x; 1])."""
    t_steps = y_core.shape[1]
    xt = y_core.transpose(2, 1, 0)  # [OBS, T, BPC]
    out = []
    for g in range(G):
        xa = np.empty((KA, t_steps, BG), np.float32)
        xa[0:OBS] = xt[:, :, g * BG:(g + 1) * BG]
        xa[OBS] = 1.0
        out.append(np.ascontiguousarray(xa.astype(BF16)))
    return out


def _prep_weights2(Wx, Wh, b):
    """wall [128, 256] bf16: cols 0:128 = block-diag Wh (rows 0:64 ->
    chunk B = [f;g], rows 64:128 -> chunk A = [i;o]); cols 128:194 rows
    0:66 = [Wx;b] for B then A."""
    H = HID
    idx_i = np.arange(0, H)
    idx_f = np.arange(H, 2 * H)
    idx_g = np.arange(2 * H, 3 * H)
    idx_o = np.arange(3 * H, 4 * H)
    A = np.concatenate([idx_i, idx_o])
    Bo = np.concatenate([idx_f, idx_g])
    Whf = np.asarray(Wh, np.float32)
    Wxa = np.concatenate([np.asarray(Wx, np.float32),
                          np.asarray(b, np.float32)[None, :]], axis=0)
    wall = np.zeros((128, 256), np.float32)
    wall[0:64, 0:128] = Whf[:, Bo]
    wall[64:128, 0:128] = Whf[:, A]
    wall[0:KA, 128:256] = Wxa[:, Bo]
    wall[KA:2 * KA, 128:256] = Wxa[:, A]
    return wall.astype(BF16)


def _prep_x2(y_core):
    """y_core [BPC, T, OBS] -> xcat [66, T, BPC] bf16 ([x;1] twice)."""
    t_steps = y_core.shape[1]
    xt = y_core.transpose(2, 1, 0)  # [OBS, T, BPC]
    xa = np.empty((2 * KA, t_steps, BPC), np.float32)
    xa[0:OBS] = xt
    xa[OBS] = 1.0
    xa[KA:KA + OBS] = xt
    xa[KA + OBS] = 1.0
    return np.ascontiguousarray(xa.astype(BF16))


def kernel(y, Wx, Wh, b):
    if os.environ.get("LSTM_V1"):
        return kernel_v1(y, Wx, Wh, b)
    from concourse.bass_utils import run_bass_kernel_spmd

    y = np.asarray(y)
    t_steps = y.shape[1]
    wall = _prep_weights2(Wx, Wh, b)
    key = ("v2", t_steps)
    if key not in _NC_CACHE:
        import concourse.mybir as mybir
        nc = build_nc2(t_steps)
        _split_waits(nc, mybir)
        _NC_CACHE[key] = nc
    nc = _NC_CACHE[key]
    in_maps = [{"wall": wall, "xcat": _prep_x2(y[c * BPC:(c + 1) * BPC])}
               for c in range(N_CORES)]
    globals()["_LAST_IN_MAPS"] = in_maps
    res = run_bass_kernel_spmd(
        nc, in_maps, core_ids=list(range(N_CORES)),
        trace=bool(int(os.environ.get("LSTM_TRACE", "0"))))
    out = np.empty((B_FULL, t_steps, HID), np.float32)
    for c in range(N_CORES):
        hg = res.results[c]["hout"].astype(np.float32)  # [HID, T, BPC]
        out[c * BPC:(c + 1) * BPC] = hg.transpose(2, 1, 0)
    globals()["_LAST_RESULT"] = res
    return out


def kernel_v1(y, Wx, Wh, b):
    from concourse.bass_utils import run_bass_kernel_spmd

    y = np.asarray(y)
    t_steps = y.shape[1]
    wcat = _prep_weights(Wx, Wh, b)

    key = t_steps
    if key not in _NC_CACHE:
        import concourse.mybir as mybir
        nc = build_nc(t_steps)
        _split_waits(nc, mybir)   # CoreSim can't run the split form
        _NC_CACHE[key] = nc
    nc = _NC_CACHE[key]

    scanc = np.zeros((HID, 2 * BG), np.float32)
    scanc[:, 1::2] = 0.5
    in_maps = []
    for c in range(N_CORES):
        xs = _prep_x(y[c * BPC:(c + 1) * BPC])
        m = {"wcat": wcat, "scanc": scanc}
        for g in range(G):
            m[f"x{g}"] = xs[g]
        in_maps.append(m)

    globals()["_LAST_IN_MAPS"] = in_maps
    res = run_bass_kernel_spmd(
        nc, in_maps, core_ids=list(range(N_CORES)),
        trace=bool(int(os.environ.get("LSTM_TRACE", "0"))))

    out = np.empty((B_FULL, t_steps, HID), np.float32)
    for c in range(N_CORES):
        for g in range(G):
            hg = res.results[c][f"h{g}"].astype(np.float32)  # [HID, T, BG]
            out[c * BPC + g * BG:c * BPC + (g + 1) * BG] = (
                hg.transpose(2, 1, 0) * 0.5)
    globals()["_LAST_RESULT"] = res
    return out

